# revision 29
# baseline (speedup 1.0000x reference)
"""MoE router kernel for Trainium2 (8 NeuronCores, SPMD data-parallel).

Problem: hidden_states [4, 4096, 2048] f32, W [2048, 64] f32, b [64] f32.
  logits = hidden @ W + b ; probs = sigmoid(logits)
  top-8 over experts -> (probs_topk normalized [B,S,8], indices [B,S,8] i32,
                         dense routing_map [B,S,64])

Sharding: tokens (batch*seq = 16384) split evenly across 8 cores (2048 each).
W/b replicated. No collectives.

Per-core pipeline (all fp32 on the PE for exact-enough top-k selection):
  - DMA x tiles [128, 2048] (tokens on partitions)
  - PE transpose 128x128 chunks -> PSUM -> copy to SBUF (h on partitions)
  - fp32 matmul: lhsT = W chunk [128h, 64e] stationary, rhs = xT [128h, 512t],
    accumulate 16 chunks into PSUM logits_T [64e, 512t]
  - ACT Identity(+bias b per-partition) PSUM->SBUF
  - PE transpose [64,128] -> [128t, 64e]; top-8 of logits == top-8 of probs
    (sigmoid monotonic) via DVE max / max_index (matches lax.top_k tie rules)
  - sigmoid on the 8 winners + full tile, normalize, threshold-scatter
"""

import numpy as np

import concourse.bass as bass
import concourse.mybir as mybir
import concourse.tile as tile
from concourse import bacc, bass_utils
from concourse.masks import make_identity

HIDDEN = 2048
NUM_EXPERTS = 64
TOPK = 8
N_CORES = 8
B, S = 4, 4096
TOKENS = B * S                      # 16384
TOK_PER_CORE = TOKENS // N_CORES    # 2048
P = 128                             # partitions / tile rows
SUP = 512                           # tokens per supertile (max fp32 moving dim)
N_SUP = TOK_PER_CORE // SUP         # 4
TT_PER_SUP = SUP // P               # 4
KC = HIDDEN // P                    # 16 contraction chunks

FP32 = mybir.dt.float32


def build_nc(loop_iters=None, ablate=None):
    """Build the per-core program. loop_iters wraps the whole body in an
    on-device For_i repeat loop; ablate in {"nomm","notr"} drops the matmuls
    or the transposes (timing analysis only -- results are garbage)."""
    nc = bacc.Bacc("TRN2", target_bir_lowering=False, debug=False,
                   num_devices=N_CORES)

    NG = N_SUP * TT_PER_SUP  # 16 token groups of 128

    # x is token-major [2048, 2048] (natural). W comes in PRE-PERMUTED on the
    # host to [128, KC*64] (partition-major) and the outputs leave the device
    # in partition-major [128, NG*K] layout (token t = g*128 + p) so every
    # DMA descriptor is a contiguous per-partition run; the host unpermutes.
    x_d = nc.dram_tensor("x", [TOK_PER_CORE, HIDDEN], FP32, kind="ExternalInput")
    w_d = nc.dram_tensor("w", [P, KC * NUM_EXPERTS], FP32, kind="ExternalInput")
    b_d = nc.dram_tensor("b", [NUM_EXPERTS], FP32, kind="ExternalInput")

    probs_d = nc.dram_tensor("probs", [P, NG * TOPK], FP32,
                             kind="ExternalOutput")
    idx_d = nc.dram_tensor("idx", [P, NG * TOPK], mybir.dt.int32,
                           kind="ExternalOutput")
    routing_d = nc.dram_tensor("routing", [P, NG * NUM_EXPERTS], FP32,
                               kind="ExternalOutput")

    with tile.TileContext(nc) as tc:
        with (
            tc.tile_pool(name="const", bufs=1) as const_pool,
            tc.tile_pool(name="xin", bufs=6) as x_pool,
            tc.tile_pool(name="xt", bufs=4) as xt_pool,
            tc.tile_pool(name="z", bufs=2) as z_pool,
            tc.tile_pool(name="zt", bufs=5) as zt_pool,
            tc.tile_pool(name="small", bufs=5) as small_pool,
            tc.tile_pool(name="acc", bufs=1) as acc_pool,
            tc.tile_pool(name="psx", bufs=3, space=bass.MemorySpace.PSUM) as psx_pool,
            tc.tile_pool(name="psl", bufs=1, space=bass.MemorySpace.PSUM) as psl_pool,
            tc.tile_pool(name="psz", bufs=1, space=bass.MemorySpace.PSUM) as psz_pool,
        ):
            ident = const_pool.tile([P, P], FP32)
            make_identity(nc, ident[:])

            w_sb = const_pool.tile([P, KC * NUM_EXPERTS], FP32)
            # bias for the split accumulator [L0; L1]: add b to L0's half only
            b_sb = const_pool.tile([P, 1], FP32)
            nc.vector.memset(b_sb[NUM_EXPERTS:P, :], 0.0)

            ps_const = None
            if ablate == "notr":
                ps_const = psx_pool.tile([P, 2 * SUP], FP32, tag="psconst")
                nc.vector.memset(ps_const[:], 0.125)

            ptop_acc = acc_pool.tile([P, NG * TOPK], FP32)
            idx_acc = acc_pool.tile([P, NG * TOPK], mybir.dt.uint32)
            rout_acc = acc_pool.tile([P, NG * NUM_EXPERTS], FP32)
            if ablate == "dmaonly":
                nc.vector.memset(ptop_acc[:], 0.0)
                nc.vector.memset(idx_acc[:].bitcast(mybir.dt.int32), 0)
                nc.vector.memset(rout_acc[:], 0.0)

            from contextlib import nullcontext
            loop_cm = (
                tc.For_i(0, loop_iters, 1,
                         hint_engines=(mybir.EngineType.PE,
                                       mybir.EngineType.Activation,
                                       mybir.EngineType.DVE,
                                       mybir.EngineType.SP))
                if loop_iters else nullcontext()
            )
            with loop_cm:
                _emit_body(nc, tc, x_d, w_d, b_d, probs_d, idx_d, routing_d,
                           ident, w_sb, b_sb, ptop_acc, idx_acc, rout_acc,
                           x_pool, xt_pool, z_pool, zt_pool, small_pool,
                           psx_pool, psl_pool, psz_pool, ablate, ps_const)

    nc.compile()
    return nc


def _emit_body(nc, tc, x_d, w_d, b_d, probs_d, idx_d, routing_d, ident, w_sb,
               b_sb, ptop_acc, idx_acc, rout_acc, x_pool, xt_pool, z_pool,
               zt_pool, small_pool, psx_pool, psl_pool, psz_pool, ablate=None,
               ps_const=None):
    NG = N_SUP * TT_PER_SUP

    def emit_topk_early(st, z_sb):
        """Phase 1 of top-k for supertile st: PE z-transposes into one merged
        PSUM bank, ACT copies out, DVE finds the top-8.  Every op here depends
        only on work that finished long ago, so nothing blocks the in-order
        ACT/DVE streams that also carry the pair copies."""
        state = {"zt": [], "v8": [], "rec": []}
        zt_ps = psz_pool.tile([P, TT_PER_SUP * P], FP32, tag="zps")
        for tt in range(TT_PER_SUP):
            nc.tensor.transpose(
                zt_ps[:, tt * P:(tt + 1) * P],
                z_sb[:, tt * P:(tt + 1) * P],
                ident[:],
            )
        zt2_sb = zt_pool.tile([P, TT_PER_SUP * P], FP32, tag="zt2")
        nc.scalar.copy(zt2_sb[:], zt_ps[:])
        for tt in range(TT_PER_SUP):
            g = st * TT_PER_SUP + tt
            # combine the split-accumulator halves: z = (L0+b)^T + L1^T
            zt_sb = zt_pool.tile([P, NUM_EXPERTS], FP32, tag="zt")
            nc.vector.tensor_add(
                zt_sb[:],
                zt2_sb[:, tt * P:tt * P + NUM_EXPERTS],
                zt2_sb[:, tt * P + NUM_EXPERTS:(tt + 1) * P],
            )
            v8 = small_pool.tile([P, TOPK], FP32, tag="v8")
            nc.vector.max(out=v8[:], in_=zt_sb[:])
            nc.vector.max_index(
                out=idx_acc[:, g * TOPK:(g + 1) * TOPK],
                in_max=v8[:],
                in_values=zt_sb[:],
            )
            state["zt"].append(zt_sb)
            state["v8"].append(v8)
        return state

    def emit_topk_late(st, state):
        """Phase 2: sigmoids (ACT) + normalize/scatter (DVE) + output flush.
        Emitted after the supertile's pair copies so the cross-engine waits
        here never delay the matmul feed chain."""
        for tt in range(TT_PER_SUP):
            g = st * TT_PER_SUP + tt
            zt_sb, v8 = state["zt"][tt], state["v8"][tt]
            p8 = small_pool.tile([P, TOPK], FP32, tag="p8")
            nc.scalar.activation(
                p8[:], v8[:], mybir.ActivationFunctionType.Sigmoid
            )
            sig_all = small_pool.tile([P, NUM_EXPERTS], FP32, tag="sig")
            nc.scalar.activation(
                sig_all[:], zt_sb[:], mybir.ActivationFunctionType.Sigmoid
            )
            s1 = small_pool.tile([P, 1], FP32, tag="s1")
            nc.vector.reduce_sum(s1[:], p8[:], axis=mybir.AxisListType.X)
            rec = small_pool.tile([P, 1], FP32, tag="rec")
            nc.vector.reciprocal(rec[:], s1[:])
            nc.vector.tensor_scalar_mul(
                ptop_acc[:, g * TOPK:(g + 1) * TOPK], p8[:], rec[:]
            )
            selrec = small_pool.tile([P, NUM_EXPERTS], FP32, tag="sel")
            nc.vector.tensor_scalar(
                selrec[:],
                zt_sb[:],
                v8[:, TOPK - 1:TOPK],
                rec[:],
                op0=mybir.AluOpType.is_ge,
                op1=mybir.AluOpType.mult,
            )
            nc.vector.tensor_mul(
                rout_acc[:, g * NUM_EXPERTS:(g + 1) * NUM_EXPERTS],
                selrec[:], sig_all[:],
            )

        # flush this supertile's outputs (keeps the kernel tail short)
        g0, g1 = st * TT_PER_SUP, (st + 1) * TT_PER_SUP
        nc.sync.dma_start(
            probs_d.ap()[:, g0 * TOPK:g1 * TOPK],
            ptop_acc[:, g0 * TOPK:g1 * TOPK],
        )
        nc.sync.dma_start(
            idx_d.ap()[:, g0 * TOPK:g1 * TOPK],
            idx_acc[:, g0 * TOPK:g1 * TOPK].bitcast(mybir.dt.int32),
        )
        nc.sync.dma_start(
            routing_d.ap()[:, g0 * NUM_EXPERTS:g1 * NUM_EXPERTS],
            rout_acc[:, g0 * NUM_EXPERTS:g1 * NUM_EXPERTS],
        )

    pending_topk = None  # (st, z_sb) of the previous supertile

    for st in range(N_SUP):
        x_tiles = []
        # x tiles hold 2 token-tiles each: [128, 2, HIDDEN]; token-tile tt
        # lives in x_tiles[tt // 2][:, tt % 2, :]
        if st == 0:
            # first supertile: load in column pieces so chunk-0 transposes can
            # start after ~1/4 of the bytes; interleave W/b right after the
            # first piece (the first matmul needs W only after 4 transposes)
            for half in range(2):
                xt_in = x_pool.tile([P, 2, HIDDEN], FP32, tag="xin")
                x_tiles.append(xt_in)
            NPIECE = 4
            pw = HIDDEN // NPIECE
            for piece in range(NPIECE):
                for half in range(2):
                    r0 = st * SUP + half * 2 * P
                    eng = nc.sync if half == 0 else nc.scalar
                    eng.dma_start(
                        x_tiles[half][:, :, piece * pw:(piece + 1) * pw],
                        x_d.ap()[r0:r0 + 2 * P, piece * pw:(piece + 1) * pw]
                        .rearrange("(t p) h -> p t h", p=P),
                    )
                if piece == 0:
                    nc.scalar.dma_start(w_sb[:], w_d.ap())
                    nc.scalar.dma_start(
                        b_sb[0:NUM_EXPERTS, :],
                        b_d.ap().rearrange("(e one) -> e one", one=1),
                    )
        else:
            # later supertiles prefetch on the ACT HWDGE ring so the sync
            # ring's sequencer stays free for output flushes
            for half in range(2):
                xt_in = x_pool.tile([P, 2, HIDDEN], FP32, tag="xin")
                r0 = st * SUP + half * 2 * P
                eng = nc.sync if half == 0 else nc.scalar
                eng.dma_start(
                    xt_in[:],
                    x_d.ap()[r0:r0 + 2 * P, :]
                    .rearrange("(t p) h -> p t h", p=P),
                )
                x_tiles.append(xt_in)

        if ablate == "dmaonly":
            g0, g1 = st * TT_PER_SUP, (st + 1) * TT_PER_SUP
            nc.sync.dma_start(
                probs_d.ap()[:, g0 * TOPK:g1 * TOPK],
                ptop_acc[:, g0 * TOPK:g1 * TOPK],
            )
            nc.sync.dma_start(
                idx_d.ap()[:, g0 * TOPK:g1 * TOPK],
                idx_acc[:, g0 * TOPK:g1 * TOPK].bitcast(mybir.dt.int32),
            )
            nc.sync.dma_start(
                routing_d.ap()[:, g0 * NUM_EXPERTS:g1 * NUM_EXPERTS],
                rout_acc[:, g0 * NUM_EXPERTS:g1 * NUM_EXPERTS],
            )
            continue

        # split accumulator: even chunks -> rows 0:64 (col group 0), odd
        # chunks -> rows 64:128 (col group 1); the two col-tiled matmuls run
        # concurrently in the PE array (~1.9x measured)
        logits_ps = psl_pool.tile([P, SUP], FP32)
        if ablate == "nomm":
            nc.vector.memset(logits_ps[:], 0.125)

        def emit_mm(c):
            if ablate == "nomm":
                return
            half = c % 2
            nc.tensor.matmul(
                logits_ps[half * NUM_EXPERTS:(half + 1) * NUM_EXPERTS, :],
                w_sb[:, c * NUM_EXPERTS:(c + 1) * NUM_EXPERTS],
                xt_done[c],
                start=(c < 2),
                stop=(c >= KC - 2),
                tile_position=(0, half * NUM_EXPERTS),
            )

        # software pipeline by chunk PAIR: 8 transposes into a 2-bank PSUM
        # tile, ONE [128,1024] copy per pair, then the two col-tiled matmuls
        # of pair p-1 back-to-back (adjacent PE instructions are required for
        # the col groups to run concurrently)
        xt_done = {}
        for p in range(KC // 2):
            if ablate == "notr":
                xt_ps = ps_const
            else:
                xt_ps = psx_pool.tile([P, 2 * SUP], FP32, tag="xt_ps")
                for ci in range(2):
                    c = 2 * p + ci
                    for tt in range(TT_PER_SUP):
                        nc.tensor.transpose(
                            xt_ps[:, ci * SUP + tt * P:ci * SUP + (tt + 1) * P],
                            x_tiles[tt // 2][:, tt % 2, c * P:(c + 1) * P],
                            ident[:],
                        )
            xt_sb = xt_pool.tile([P, 2 * SUP], FP32, tag="xt")
            if p % 2 == 0:
                nc.scalar.copy(xt_sb[:], xt_ps[:])
            else:
                nc.vector.tensor_copy(xt_sb[:], xt_ps[:])
            xt_done[2 * p] = xt_sb[:, 0:SUP]
            xt_done[2 * p + 1] = xt_sb[:, SUP:2 * SUP]
            if p >= 1:
                emit_mm(2 * p - 2)
                emit_mm(2 * p - 1)
            if p == 1 and pending_topk is not None:
                topk_state = emit_topk_early(*pending_topk)

        emit_mm(KC - 2)
        emit_mm(KC - 1)
        if pending_topk is not None:
            emit_topk_late(pending_topk[0], topk_state)
            pending_topk = None

        # zz = [L0 + b ; L1], both halves on partitions  [128, 512]
        z_sb = z_pool.tile([P, SUP], FP32, tag="z")
        nc.scalar.activation(
            z_sb[:], logits_ps[:],
            mybir.ActivationFunctionType.Identity, bias=b_sb[:],
        )
        pending_topk = (st, z_sb)

    if pending_topk is not None:
        topk_state = emit_topk_early(*pending_topk)
        emit_topk_late(pending_topk[0], topk_state)


_NC_CACHE = None


def _get_nc():
    global _NC_CACHE
    if _NC_CACHE is None:
        _NC_CACHE = build_nc()
    return _NC_CACHE


def run_sharded(hidden_states, W, b, trace=False):
    nc = _get_nc()
    hs = np.ascontiguousarray(np.asarray(hidden_states, dtype=np.float32))
    W = np.asarray(W, dtype=np.float32)
    b = np.ascontiguousarray(np.asarray(b, dtype=np.float32))
    # device wants W as [128, KC*64]: W[h=c*128+p, e] -> w_perm[p, c*64+e]
    w_perm = np.ascontiguousarray(
        W.reshape(KC, P, NUM_EXPERTS).transpose(1, 0, 2).reshape(P, KC * NUM_EXPERTS)
    )
    flat = hs.reshape(TOKENS, HIDDEN)
    in_maps = [
        {
            "x": flat[c * TOK_PER_CORE:(c + 1) * TOK_PER_CORE],
            "w": w_perm,
            "b": b,
        }
        for c in range(N_CORES)
    ]
    out = bass_utils.run_bass_kernel_spmd(
        nc, in_maps, core_ids=list(range(N_CORES)), trace=trace
    )
    return out


def _unperm(a, width):
    """[128, NG*width] partition-major -> [TOK_PER_CORE, width] token-major."""
    ng = a.shape[1] // width
    return a.reshape(P, ng, width).transpose(1, 0, 2).reshape(P * ng, width)


def kernel(hidden_states, W, b):
    out = run_sharded(hidden_states, W, b)
    res = out.results
    probs = np.concatenate([_unperm(r["probs"], TOPK) for r in res], axis=0)
    idx = np.concatenate([_unperm(r["idx"], TOPK) for r in res], axis=0)
    routing = np.concatenate(
        [_unperm(r["routing"], NUM_EXPERTS) for r in res], axis=0
    )
    probs_topk = probs.reshape(B, S, TOPK).astype(np.float32)
    indices_topk = idx.reshape(B, S, TOPK).astype(np.int32)
    routing_map = routing.reshape(B, S, NUM_EXPERTS).astype(np.float32)
    return probs_topk, indices_topk, routing_map


# revision 30
# speedup vs baseline: 1.0562x; 1.0562x over previous
"""MoE router kernel for Trainium2 (8 NeuronCores, SPMD data-parallel).

Problem: hidden_states [4, 4096, 2048] f32, W [2048, 64] f32, b [64] f32.
  logits = hidden @ W + b ; probs = sigmoid(logits)
  top-8 over experts -> (probs_topk normalized [B,S,8], indices [B,S,8] i32,
                         dense routing_map [B,S,64])

Sharding: tokens (batch*seq = 16384) split evenly across 8 cores (2048 each).
W/b replicated. No collectives.

Per-core pipeline (all fp32 on the PE for exact-enough top-k selection):
  - DMA x tiles [128, 2048] (tokens on partitions)
  - PE transpose 128x128 chunks -> PSUM -> copy to SBUF (h on partitions)
  - fp32 matmul: lhsT = W chunk [128h, 64e] stationary, rhs = xT [128h, 512t],
    accumulate 16 chunks into PSUM logits_T [64e, 512t]
  - ACT Identity(+bias b per-partition) PSUM->SBUF
  - PE transpose [64,128] -> [128t, 64e]; top-8 of logits == top-8 of probs
    (sigmoid monotonic) via DVE max / max_index (matches lax.top_k tie rules)
  - sigmoid on the 8 winners + full tile, normalize, threshold-scatter
"""

import numpy as np

import concourse.bass as bass
import concourse.mybir as mybir
import concourse.tile as tile
from concourse import bacc, bass_utils
from concourse.masks import make_identity

HIDDEN = 2048
NUM_EXPERTS = 64
TOPK = 8
N_CORES = 8
B, S = 4, 4096
TOKENS = B * S                      # 16384
TOK_PER_CORE = TOKENS // N_CORES    # 2048
P = 128                             # partitions / tile rows
SUP = 512                           # tokens per supertile (max fp32 moving dim)
N_SUP = TOK_PER_CORE // SUP         # 4
TT_PER_SUP = SUP // P               # 4
KC = HIDDEN // P                    # 16 contraction chunks

FP32 = mybir.dt.float32


def build_nc(loop_iters=None, ablate=None):
    """Build the per-core program. loop_iters wraps the whole body in an
    on-device For_i repeat loop; ablate in {"nomm","notr"} drops the matmuls
    or the transposes (timing analysis only -- results are garbage)."""
    nc = bacc.Bacc("TRN2", target_bir_lowering=False, debug=False,
                   num_devices=N_CORES)

    NG = N_SUP * TT_PER_SUP  # 16 token groups of 128

    # x is token-major [2048, 2048] (natural). W comes in PRE-PERMUTED on the
    # host to [128, KC*64] (partition-major) and the outputs leave the device
    # in partition-major [128, NG*K] layout (token t = g*128 + p) so every
    # DMA descriptor is a contiguous per-partition run; the host unpermutes.
    x_d = nc.dram_tensor("x", [TOK_PER_CORE, HIDDEN], FP32, kind="ExternalInput")
    w_d = nc.dram_tensor("w", [P, KC * NUM_EXPERTS], FP32, kind="ExternalInput")
    b_d = nc.dram_tensor("b", [NUM_EXPERTS], FP32, kind="ExternalInput")

    probs_d = nc.dram_tensor("probs", [P, NG * TOPK], FP32,
                             kind="ExternalOutput")
    idx_d = nc.dram_tensor("idx", [P, NG * TOPK], mybir.dt.int32,
                           kind="ExternalOutput")
    routing_d = nc.dram_tensor("routing", [P, NG * NUM_EXPERTS], FP32,
                               kind="ExternalOutput")

    with tile.TileContext(nc) as tc:
        with (
            tc.tile_pool(name="const", bufs=1) as const_pool,
            tc.tile_pool(name="xin", bufs=6) as x_pool,
            tc.tile_pool(name="xt", bufs=4) as xt_pool,
            tc.tile_pool(name="z", bufs=2) as z_pool,
            tc.tile_pool(name="zt", bufs=5) as zt_pool,
            tc.tile_pool(name="small", bufs=5) as small_pool,
            tc.tile_pool(name="acc", bufs=1) as acc_pool,
            tc.tile_pool(name="psx", bufs=6, space=bass.MemorySpace.PSUM) as psx_pool,
            tc.tile_pool(name="psl", bufs=1, space=bass.MemorySpace.PSUM) as psl_pool,
            tc.tile_pool(name="psz", bufs=1, space=bass.MemorySpace.PSUM) as psz_pool,
        ):
            ident = const_pool.tile([P, P], FP32)
            make_identity(nc, ident[:])

            w_sb = const_pool.tile([P, KC * NUM_EXPERTS], FP32)
            # bias for the split accumulator [L0; L1]: add b to L0's half only
            b_sb = const_pool.tile([P, 1], FP32)
            nc.vector.memset(b_sb[NUM_EXPERTS:P, :], 0.0)

            ps_const = None
            if ablate == "notr":
                ps_const = psx_pool.tile([P, SUP], FP32, tag="psconst")
                nc.vector.memset(ps_const[:], 0.125)

            ptop_acc = acc_pool.tile([P, NG * TOPK], FP32)
            idx_acc = acc_pool.tile([P, NG * TOPK], mybir.dt.uint32)
            rout_acc = acc_pool.tile([P, NG * NUM_EXPERTS], FP32)
            if ablate == "dmaonly":
                nc.vector.memset(ptop_acc[:], 0.0)
                nc.vector.memset(idx_acc[:].bitcast(mybir.dt.int32), 0)
                nc.vector.memset(rout_acc[:], 0.0)

            from contextlib import nullcontext
            loop_cm = (
                tc.For_i(0, loop_iters, 1,
                         hint_engines=(mybir.EngineType.PE,
                                       mybir.EngineType.Activation,
                                       mybir.EngineType.DVE,
                                       mybir.EngineType.SP))
                if loop_iters else nullcontext()
            )
            with loop_cm:
                _emit_body(nc, tc, x_d, w_d, b_d, probs_d, idx_d, routing_d,
                           ident, w_sb, b_sb, ptop_acc, idx_acc, rout_acc,
                           x_pool, xt_pool, z_pool, zt_pool, small_pool,
                           psx_pool, psl_pool, psz_pool, ablate, ps_const)

    nc.compile()
    return nc


def _emit_body(nc, tc, x_d, w_d, b_d, probs_d, idx_d, routing_d, ident, w_sb,
               b_sb, ptop_acc, idx_acc, rout_acc, x_pool, xt_pool, z_pool,
               zt_pool, small_pool, psx_pool, psl_pool, psz_pool, ablate=None,
               ps_const=None):
    NG = N_SUP * TT_PER_SUP

    def emit_topk_early(st, z_sb):
        """Phase 1 of top-k for supertile st: PE z-transposes into one merged
        PSUM bank, ACT copies out, DVE finds the top-8.  Every op here depends
        only on work that finished long ago, so nothing blocks the in-order
        ACT/DVE streams that also carry the pair copies."""
        state = {"zt": [], "v8": [], "rec": []}
        zt_ps = psz_pool.tile([P, TT_PER_SUP * P], FP32, tag="zps")
        for tt in range(TT_PER_SUP):
            nc.tensor.transpose(
                zt_ps[:, tt * P:(tt + 1) * P],
                z_sb[:, tt * P:(tt + 1) * P],
                ident[:],
            )
        zt2_sb = zt_pool.tile([P, TT_PER_SUP * P], FP32, tag="zt2")
        nc.scalar.copy(zt2_sb[:], zt_ps[:])
        for tt in range(TT_PER_SUP):
            g = st * TT_PER_SUP + tt
            # combine the split-accumulator halves: z = (L0+b)^T + L1^T
            zt_sb = zt_pool.tile([P, NUM_EXPERTS], FP32, tag="zt")
            nc.vector.tensor_add(
                zt_sb[:],
                zt2_sb[:, tt * P:tt * P + NUM_EXPERTS],
                zt2_sb[:, tt * P + NUM_EXPERTS:(tt + 1) * P],
            )
            v8 = small_pool.tile([P, TOPK], FP32, tag="v8")
            nc.vector.max(out=v8[:], in_=zt_sb[:])
            nc.vector.max_index(
                out=idx_acc[:, g * TOPK:(g + 1) * TOPK],
                in_max=v8[:],
                in_values=zt_sb[:],
            )
            state["zt"].append(zt_sb)
            state["v8"].append(v8)
        return state

    def emit_topk_late(st, state):
        """Phase 2: sigmoids (ACT) + normalize/scatter (DVE) + output flush.
        Emitted after the supertile's pair copies so the cross-engine waits
        here never delay the matmul feed chain."""
        for tt in range(TT_PER_SUP):
            g = st * TT_PER_SUP + tt
            zt_sb, v8 = state["zt"][tt], state["v8"][tt]
            p8 = small_pool.tile([P, TOPK], FP32, tag="p8")
            nc.scalar.activation(
                p8[:], v8[:], mybir.ActivationFunctionType.Sigmoid
            )
            sig_all = small_pool.tile([P, NUM_EXPERTS], FP32, tag="sig")
            nc.scalar.activation(
                sig_all[:], zt_sb[:], mybir.ActivationFunctionType.Sigmoid
            )
            s1 = small_pool.tile([P, 1], FP32, tag="s1")
            nc.vector.reduce_sum(s1[:], p8[:], axis=mybir.AxisListType.X)
            rec = small_pool.tile([P, 1], FP32, tag="rec")
            nc.vector.reciprocal(rec[:], s1[:])
            nc.vector.tensor_scalar_mul(
                ptop_acc[:, g * TOPK:(g + 1) * TOPK], p8[:], rec[:]
            )
            selrec = small_pool.tile([P, NUM_EXPERTS], FP32, tag="sel")
            nc.vector.tensor_scalar(
                selrec[:],
                zt_sb[:],
                v8[:, TOPK - 1:TOPK],
                rec[:],
                op0=mybir.AluOpType.is_ge,
                op1=mybir.AluOpType.mult,
            )
            nc.vector.tensor_mul(
                rout_acc[:, g * NUM_EXPERTS:(g + 1) * NUM_EXPERTS],
                selrec[:], sig_all[:],
            )

        # flush this supertile's outputs (keeps the kernel tail short)
        g0, g1 = st * TT_PER_SUP, (st + 1) * TT_PER_SUP
        nc.sync.dma_start(
            probs_d.ap()[:, g0 * TOPK:g1 * TOPK],
            ptop_acc[:, g0 * TOPK:g1 * TOPK],
        )
        nc.sync.dma_start(
            idx_d.ap()[:, g0 * TOPK:g1 * TOPK],
            idx_acc[:, g0 * TOPK:g1 * TOPK].bitcast(mybir.dt.int32),
        )
        nc.sync.dma_start(
            routing_d.ap()[:, g0 * NUM_EXPERTS:g1 * NUM_EXPERTS],
            rout_acc[:, g0 * NUM_EXPERTS:g1 * NUM_EXPERTS],
        )

    pending_topk = None  # (st, z_sb) of the previous supertile

    for st in range(N_SUP):
        x_tiles = []
        # x tiles hold 2 token-tiles each: [128, 2, HIDDEN]; token-tile tt
        # lives in x_tiles[tt // 2][:, tt % 2, :]
        if st == 0:
            # first supertile: load in column pieces so chunk-0 transposes can
            # start after ~1/4 of the bytes; interleave W/b right after the
            # first piece (the first matmul needs W only after 4 transposes)
            for half in range(2):
                xt_in = x_pool.tile([P, 2, HIDDEN], FP32, tag="xin")
                x_tiles.append(xt_in)
            NPIECE = 4
            pw = HIDDEN // NPIECE
            for piece in range(NPIECE):
                for half in range(2):
                    r0 = st * SUP + half * 2 * P
                    eng = nc.sync if half == 0 else nc.scalar
                    eng.dma_start(
                        x_tiles[half][:, :, piece * pw:(piece + 1) * pw],
                        x_d.ap()[r0:r0 + 2 * P, piece * pw:(piece + 1) * pw]
                        .rearrange("(t p) h -> p t h", p=P),
                    )
                if piece == 0:
                    nc.scalar.dma_start(w_sb[:], w_d.ap())
                    nc.scalar.dma_start(
                        b_sb[0:NUM_EXPERTS, :],
                        b_d.ap().rearrange("(e one) -> e one", one=1),
                    )
        else:
            # later supertiles prefetch on the ACT HWDGE ring so the sync
            # ring's sequencer stays free for output flushes
            for half in range(2):
                xt_in = x_pool.tile([P, 2, HIDDEN], FP32, tag="xin")
                r0 = st * SUP + half * 2 * P
                eng = nc.sync if half == 0 else nc.scalar
                eng.dma_start(
                    xt_in[:],
                    x_d.ap()[r0:r0 + 2 * P, :]
                    .rearrange("(t p) h -> p t h", p=P),
                )
                x_tiles.append(xt_in)

        if ablate == "dmaonly":
            g0, g1 = st * TT_PER_SUP, (st + 1) * TT_PER_SUP
            nc.sync.dma_start(
                probs_d.ap()[:, g0 * TOPK:g1 * TOPK],
                ptop_acc[:, g0 * TOPK:g1 * TOPK],
            )
            nc.sync.dma_start(
                idx_d.ap()[:, g0 * TOPK:g1 * TOPK],
                idx_acc[:, g0 * TOPK:g1 * TOPK].bitcast(mybir.dt.int32),
            )
            nc.sync.dma_start(
                routing_d.ap()[:, g0 * NUM_EXPERTS:g1 * NUM_EXPERTS],
                rout_acc[:, g0 * NUM_EXPERTS:g1 * NUM_EXPERTS],
            )
            continue

        # split accumulator: even chunks -> rows 0:64 (col group 0), odd
        # chunks -> rows 64:128 (col group 1); the two col-tiled matmuls run
        # concurrently in the PE array (~1.9x measured)
        logits_ps = psl_pool.tile([P, SUP], FP32)
        if ablate == "nomm":
            nc.vector.memset(logits_ps[:], 0.125)

        def emit_mm(c):
            if ablate == "nomm":
                return
            half = c % 2
            nc.tensor.matmul(
                logits_ps[half * NUM_EXPERTS:(half + 1) * NUM_EXPERTS, :],
                w_sb[:, c * NUM_EXPERTS:(c + 1) * NUM_EXPERTS],
                xt_done[c],
                start=(c < 2),
                stop=(c >= KC - 2),
                tile_position=(0, half * NUM_EXPERTS),
            )

        # software pipeline by chunk PAIR: 8 transposes into a 2-bank PSUM
        # tile, ONE [128,1024] copy per pair, then the two col-tiled matmuls
        # of pair p-1 back-to-back (adjacent PE instructions are required for
        # the col groups to run concurrently)
        xt_done = {}
        for p in range(KC // 2):
            for ci in range(2):
                c = 2 * p + ci
                if ablate == "notr":
                    xt_ps = ps_const
                else:
                    xt_ps = psx_pool.tile([P, SUP], FP32, tag="xt_ps")
                    for tt in range(TT_PER_SUP):
                        nc.tensor.transpose(
                            xt_ps[:, tt * P:(tt + 1) * P],
                            x_tiles[tt // 2][:, tt % 2, c * P:(c + 1) * P],
                            ident[:],
                        )
                xt_sb = xt_pool.tile([P, SUP], FP32, tag="xt")
                # the two copies of a pair run on different engines in parallel
                if ci == 0:
                    nc.scalar.copy(xt_sb[:], xt_ps[:])
                else:
                    nc.vector.tensor_copy(xt_sb[:], xt_ps[:])
                xt_done[c] = xt_sb[:]
            if p >= 1:
                emit_mm(2 * p - 2)
                emit_mm(2 * p - 1)
            if p == 1 and pending_topk is not None:
                topk_state = emit_topk_early(*pending_topk)

        emit_mm(KC - 2)
        emit_mm(KC - 1)
        if pending_topk is not None:
            emit_topk_late(pending_topk[0], topk_state)
            pending_topk = None

        # zz = [L0 + b ; L1], both halves on partitions  [128, 512]
        z_sb = z_pool.tile([P, SUP], FP32, tag="z")
        nc.scalar.activation(
            z_sb[:], logits_ps[:],
            mybir.ActivationFunctionType.Identity, bias=b_sb[:],
        )
        pending_topk = (st, z_sb)

    if pending_topk is not None:
        topk_state = emit_topk_early(*pending_topk)
        emit_topk_late(pending_topk[0], topk_state)


_NC_CACHE = None


def _get_nc():
    global _NC_CACHE
    if _NC_CACHE is None:
        _NC_CACHE = build_nc()
    return _NC_CACHE


def run_sharded(hidden_states, W, b, trace=False):
    nc = _get_nc()
    hs = np.ascontiguousarray(np.asarray(hidden_states, dtype=np.float32))
    W = np.asarray(W, dtype=np.float32)
    b = np.ascontiguousarray(np.asarray(b, dtype=np.float32))
    # device wants W as [128, KC*64]: W[h=c*128+p, e] -> w_perm[p, c*64+e]
    w_perm = np.ascontiguousarray(
        W.reshape(KC, P, NUM_EXPERTS).transpose(1, 0, 2).reshape(P, KC * NUM_EXPERTS)
    )
    flat = hs.reshape(TOKENS, HIDDEN)
    in_maps = [
        {
            "x": flat[c * TOK_PER_CORE:(c + 1) * TOK_PER_CORE],
            "w": w_perm,
            "b": b,
        }
        for c in range(N_CORES)
    ]
    out = bass_utils.run_bass_kernel_spmd(
        nc, in_maps, core_ids=list(range(N_CORES)), trace=trace
    )
    return out


def _unperm(a, width):
    """[128, NG*width] partition-major -> [TOK_PER_CORE, width] token-major."""
    ng = a.shape[1] // width
    return a.reshape(P, ng, width).transpose(1, 0, 2).reshape(P * ng, width)


def kernel(hidden_states, W, b):
    out = run_sharded(hidden_states, W, b)
    res = out.results
    probs = np.concatenate([_unperm(r["probs"], TOPK) for r in res], axis=0)
    idx = np.concatenate([_unperm(r["idx"], TOPK) for r in res], axis=0)
    routing = np.concatenate(
        [_unperm(r["routing"], NUM_EXPERTS) for r in res], axis=0
    )
    probs_topk = probs.reshape(B, S, TOPK).astype(np.float32)
    indices_topk = idx.reshape(B, S, TOPK).astype(np.int32)
    routing_map = routing.reshape(B, S, NUM_EXPERTS).astype(np.float32)
    return probs_topk, indices_topk, routing_map


# revision 31
# speedup vs baseline: 1.1215x; 1.0619x over previous
"""MoE router kernel for Trainium2 (8 NeuronCores, SPMD data-parallel).

Problem: hidden_states [4, 4096, 2048] f32, W [2048, 64] f32, b [64] f32.
  logits = hidden @ W + b ; probs = sigmoid(logits)
  top-8 over experts -> (probs_topk normalized [B,S,8], indices [B,S,8] i32,
                         dense routing_map [B,S,64])

Sharding: tokens (batch*seq = 16384) split evenly across 8 cores (2048 each).
W/b replicated. No collectives.

Per-core pipeline (all fp32 on the PE for exact-enough top-k selection):
  - DMA x tiles [128, 2048] (tokens on partitions)
  - PE transpose 128x128 chunks -> PSUM -> copy to SBUF (h on partitions)
  - fp32 matmul: lhsT = W chunk [128h, 64e] stationary, rhs = xT [128h, 512t],
    accumulate 16 chunks into PSUM logits_T [64e, 512t]
  - ACT Identity(+bias b per-partition) PSUM->SBUF
  - PE transpose [64,128] -> [128t, 64e]; top-8 of logits == top-8 of probs
    (sigmoid monotonic) via DVE max / max_index (matches lax.top_k tie rules)
  - sigmoid on the 8 winners + full tile, normalize, threshold-scatter
"""

import numpy as np

import concourse.bass as bass
import concourse.mybir as mybir
import concourse.tile as tile
from concourse import bacc, bass_utils
from concourse.masks import make_identity

HIDDEN = 2048
NUM_EXPERTS = 64
TOPK = 8
N_CORES = 8
B, S = 4, 4096
TOKENS = B * S                      # 16384
TOK_PER_CORE = TOKENS // N_CORES    # 2048
P = 128                             # partitions / tile rows
SUP = 512                           # tokens per supertile (max fp32 moving dim)
N_SUP = TOK_PER_CORE // SUP         # 4
TT_PER_SUP = SUP // P               # 4
KC = HIDDEN // P                    # 16 contraction chunks

FP32 = mybir.dt.float32


def build_nc(loop_iters=None, ablate=None):
    """Build the per-core program. loop_iters wraps the whole body in an
    on-device For_i repeat loop; ablate in {"nomm","notr"} drops the matmuls
    or the transposes (timing analysis only -- results are garbage)."""
    nc = bacc.Bacc("TRN2", target_bir_lowering=False, debug=False,
                   num_devices=N_CORES)

    NG = N_SUP * TT_PER_SUP  # 16 token groups of 128

    # x is token-major [2048, 2048] (natural). W comes in PRE-PERMUTED on the
    # host to [128, KC*64] (partition-major) and the outputs leave the device
    # in partition-major [128, NG*K] layout (token t = g*128 + p) so every
    # DMA descriptor is a contiguous per-partition run; the host unpermutes.
    x_d = nc.dram_tensor("x", [TOK_PER_CORE, HIDDEN], FP32, kind="ExternalInput")
    w_d = nc.dram_tensor("w", [P, KC * NUM_EXPERTS], FP32, kind="ExternalInput")
    b_d = nc.dram_tensor("b", [NUM_EXPERTS], FP32, kind="ExternalInput")

    probs_d = nc.dram_tensor("probs", [P, NG * TOPK], FP32,
                             kind="ExternalOutput")
    idx_d = nc.dram_tensor("idx", [P, NG * TOPK], mybir.dt.int32,
                           kind="ExternalOutput")
    routing_d = nc.dram_tensor("routing", [P, NG * NUM_EXPERTS], FP32,
                               kind="ExternalOutput")

    with tile.TileContext(nc) as tc:
        with (
            tc.tile_pool(name="const", bufs=1) as const_pool,
            tc.tile_pool(name="xin", bufs=6) as x_pool,
            tc.tile_pool(name="xt", bufs=4) as xt_pool,
            tc.tile_pool(name="z", bufs=2) as z_pool,
            tc.tile_pool(name="zt", bufs=5) as zt_pool,
            tc.tile_pool(name="small", bufs=5) as small_pool,
            tc.tile_pool(name="acc", bufs=1) as acc_pool,
            tc.tile_pool(name="psx", bufs=6, space=bass.MemorySpace.PSUM) as psx_pool,
            tc.tile_pool(name="psl", bufs=1, space=bass.MemorySpace.PSUM) as psl_pool,
            tc.tile_pool(name="psz", bufs=1, space=bass.MemorySpace.PSUM) as psz_pool,
        ):
            ident = const_pool.tile([P, P], FP32)
            make_identity(nc, ident[:])

            w_sb = const_pool.tile([P, KC * NUM_EXPERTS], FP32)
            # bias for the split accumulator [L0; L1]: add b to L0's half only
            b_sb = const_pool.tile([P, 1], FP32)
            nc.vector.memset(b_sb[NUM_EXPERTS:P, :], 0.0)

            ps_const = None
            if ablate == "notr":
                ps_const = psx_pool.tile([P, SUP], FP32, tag="psconst")
                nc.vector.memset(ps_const[:], 0.125)

            ptop_acc = acc_pool.tile([P, NG * TOPK], FP32)
            idx_acc = acc_pool.tile([P, NG * TOPK], mybir.dt.uint32)
            rout_acc = acc_pool.tile([P, NG * NUM_EXPERTS], FP32)
            if ablate == "dmaonly":
                nc.vector.memset(ptop_acc[:], 0.0)
                nc.vector.memset(idx_acc[:].bitcast(mybir.dt.int32), 0)
                nc.vector.memset(rout_acc[:], 0.0)

            from contextlib import nullcontext
            loop_cm = (
                tc.For_i(0, loop_iters, 1,
                         hint_engines=(mybir.EngineType.PE,
                                       mybir.EngineType.Activation,
                                       mybir.EngineType.DVE,
                                       mybir.EngineType.SP))
                if loop_iters else nullcontext()
            )
            with loop_cm:
                _emit_body(nc, tc, x_d, w_d, b_d, probs_d, idx_d, routing_d,
                           ident, w_sb, b_sb, ptop_acc, idx_acc, rout_acc,
                           x_pool, xt_pool, z_pool, zt_pool, small_pool,
                           psx_pool, psl_pool, psz_pool, ablate, ps_const)

    nc.compile()
    return nc


def _emit_body(nc, tc, x_d, w_d, b_d, probs_d, idx_d, routing_d, ident, w_sb,
               b_sb, ptop_acc, idx_acc, rout_acc, x_pool, xt_pool, z_pool,
               zt_pool, small_pool, psx_pool, psl_pool, psz_pool, ablate=None,
               ps_const=None):
    NG = N_SUP * TT_PER_SUP

    def emit_topk_early(st, z_sb):
        """Phase 1 of top-k for supertile st: PE z-transposes into one merged
        PSUM bank, ACT copies out, DVE finds the top-8.  Every op here depends
        only on work that finished long ago, so nothing blocks the in-order
        ACT/DVE streams that also carry the pair copies."""
        state = {"zt": [], "v8": [], "rec": []}
        zt_ps = psz_pool.tile([P, TT_PER_SUP * P], FP32, tag="zps")
        for tt in range(TT_PER_SUP):
            nc.tensor.transpose(
                zt_ps[:, tt * P:(tt + 1) * P],
                z_sb[:, tt * P:(tt + 1) * P],
                ident[:],
            )
        zt2_sb = zt_pool.tile([P, TT_PER_SUP * P], FP32, tag="zt2")
        nc.scalar.copy(zt2_sb[:], zt_ps[:])
        for tt in range(TT_PER_SUP):
            g = st * TT_PER_SUP + tt
            # combine the split-accumulator halves: z = (L0+b)^T + L1^T
            zt_sb = zt_pool.tile([P, NUM_EXPERTS], FP32, tag="zt")
            nc.vector.tensor_add(
                zt_sb[:],
                zt2_sb[:, tt * P:tt * P + NUM_EXPERTS],
                zt2_sb[:, tt * P + NUM_EXPERTS:(tt + 1) * P],
            )
            v8 = small_pool.tile([P, TOPK], FP32, tag="v8")
            nc.vector.max(out=v8[:], in_=zt_sb[:])
            nc.vector.max_index(
                out=idx_acc[:, g * TOPK:(g + 1) * TOPK],
                in_max=v8[:],
                in_values=zt_sb[:],
            )
            state["zt"].append(zt_sb)
            state["v8"].append(v8)
        return state

    def emit_topk_late(st, state):
        """Phase 2: sigmoids (ACT) + normalize/scatter (DVE) + output flush.
        Emitted after the supertile's pair copies so the cross-engine waits
        here never delay the matmul feed chain."""
        for tt in range(TT_PER_SUP):
            g = st * TT_PER_SUP + tt
            zt_sb, v8 = state["zt"][tt], state["v8"][tt]
            p8 = small_pool.tile([P, TOPK], FP32, tag="p8")
            nc.scalar.activation(
                p8[:], v8[:], mybir.ActivationFunctionType.Sigmoid
            )
            sig_all = small_pool.tile([P, NUM_EXPERTS], FP32, tag="sig")
            nc.scalar.activation(
                sig_all[:], zt_sb[:], mybir.ActivationFunctionType.Sigmoid
            )
            s1 = small_pool.tile([P, 1], FP32, tag="s1")
            nc.vector.reduce_sum(s1[:], p8[:], axis=mybir.AxisListType.X)
            rec = small_pool.tile([P, 1], FP32, tag="rec")
            nc.vector.reciprocal(rec[:], s1[:])
            nc.vector.tensor_scalar_mul(
                ptop_acc[:, g * TOPK:(g + 1) * TOPK], p8[:], rec[:]
            )
            selrec = small_pool.tile([P, NUM_EXPERTS], FP32, tag="sel")
            nc.vector.tensor_scalar(
                selrec[:],
                zt_sb[:],
                v8[:, TOPK - 1:TOPK],
                rec[:],
                op0=mybir.AluOpType.is_ge,
                op1=mybir.AluOpType.mult,
            )
            nc.vector.tensor_mul(
                rout_acc[:, g * NUM_EXPERTS:(g + 1) * NUM_EXPERTS],
                selrec[:], sig_all[:],
            )

        # flush this supertile's outputs (keeps the kernel tail short)
        g0, g1 = st * TT_PER_SUP, (st + 1) * TT_PER_SUP
        nc.sync.dma_start(
            probs_d.ap()[:, g0 * TOPK:g1 * TOPK],
            ptop_acc[:, g0 * TOPK:g1 * TOPK],
        )
        nc.sync.dma_start(
            idx_d.ap()[:, g0 * TOPK:g1 * TOPK],
            idx_acc[:, g0 * TOPK:g1 * TOPK].bitcast(mybir.dt.int32),
        )
        nc.sync.dma_start(
            routing_d.ap()[:, g0 * NUM_EXPERTS:g1 * NUM_EXPERTS],
            rout_acc[:, g0 * NUM_EXPERTS:g1 * NUM_EXPERTS],
        )

    pending_topk = None  # (st, z_sb) of the previous supertile

    for st in range(N_SUP):
        x_tiles = []
        # x tiles hold 2 token-tiles each: [128, 2, HIDDEN]; token-tile tt
        # lives in x_tiles[tt // 2][:, tt % 2, :]
        if st == 0:
            # first supertile: load in column pieces so chunk-0 transposes can
            # start after ~1/4 of the bytes; interleave W/b right after the
            # first piece (the first matmul needs W only after 4 transposes)
            for half in range(2):
                xt_in = x_pool.tile([P, 2, HIDDEN], FP32, tag="xin")
                x_tiles.append(xt_in)
            NPIECE = 4
            pw = HIDDEN // NPIECE
            for piece in range(NPIECE):
                for half in range(2):
                    r0 = st * SUP + half * 2 * P
                    eng = nc.sync if half == 0 else nc.scalar
                    eng.dma_start(
                        x_tiles[half][:, :, piece * pw:(piece + 1) * pw],
                        x_d.ap()[r0:r0 + 2 * P, piece * pw:(piece + 1) * pw]
                        .rearrange("(t p) h -> p t h", p=P),
                    )
                if piece == 0:
                    nc.scalar.dma_start(w_sb[:], w_d.ap())
                    nc.scalar.dma_start(
                        b_sb[0:NUM_EXPERTS, :],
                        b_d.ap().rearrange("(e one) -> e one", one=1),
                    )
        else:
            # later supertiles prefetch on the ACT HWDGE ring so the sync
            # ring's sequencer stays free for output flushes
            for half in range(2):
                xt_in = x_pool.tile([P, 2, HIDDEN], FP32, tag="xin")
                r0 = st * SUP + half * 2 * P
                eng = nc.sync if half == 0 else nc.scalar
                eng.dma_start(
                    xt_in[:],
                    x_d.ap()[r0:r0 + 2 * P, :]
                    .rearrange("(t p) h -> p t h", p=P),
                )
                x_tiles.append(xt_in)

        if ablate == "dmaonly":
            g0, g1 = st * TT_PER_SUP, (st + 1) * TT_PER_SUP
            nc.sync.dma_start(
                probs_d.ap()[:, g0 * TOPK:g1 * TOPK],
                ptop_acc[:, g0 * TOPK:g1 * TOPK],
            )
            nc.sync.dma_start(
                idx_d.ap()[:, g0 * TOPK:g1 * TOPK],
                idx_acc[:, g0 * TOPK:g1 * TOPK].bitcast(mybir.dt.int32),
            )
            nc.sync.dma_start(
                routing_d.ap()[:, g0 * NUM_EXPERTS:g1 * NUM_EXPERTS],
                rout_acc[:, g0 * NUM_EXPERTS:g1 * NUM_EXPERTS],
            )
            continue

        # split accumulator: even chunks -> rows 0:64 (col group 0), odd
        # chunks -> rows 64:128 (col group 1); the two col-tiled matmuls run
        # concurrently in the PE array (~1.9x measured)
        logits_ps = psl_pool.tile([P, SUP], FP32)
        if ablate == "nomm":
            nc.vector.memset(logits_ps[:], 0.125)

        def emit_mm(c):
            if ablate == "nomm":
                return
            half = c % 2
            nc.tensor.matmul(
                logits_ps[half * NUM_EXPERTS:(half + 1) * NUM_EXPERTS, :],
                w_sb[:, c * NUM_EXPERTS:(c + 1) * NUM_EXPERTS],
                xt_done[c],
                start=(c < 2),
                stop=(c >= KC - 2),
                tile_position=(0, half * NUM_EXPERTS),
            )

        # software pipeline by chunk PAIR: 8 transposes into a 2-bank PSUM
        # tile, ONE [128,1024] copy per pair, then the two col-tiled matmuls
        # of pair p-1 back-to-back (adjacent PE instructions are required for
        # the col groups to run concurrently)
        xt_done = {}
        for p in range(KC // 2):
            for ci in range(2):
                c = 2 * p + ci
                if ablate == "notr":
                    xt_ps = ps_const
                else:
                    xt_ps = psx_pool.tile([P, SUP], FP32, tag="xt_ps")
                    for tt in range(TT_PER_SUP):
                        nc.tensor.transpose(
                            xt_ps[:, tt * P:(tt + 1) * P],
                            x_tiles[tt // 2][:, tt % 2, c * P:(c + 1) * P],
                            ident[:],
                        )
                xt_sb = xt_pool.tile([P, SUP], FP32, tag="xt")
                # both engines copy one half each -> half the chain latency
                nc.scalar.copy(xt_sb[:, 0:SUP // 2], xt_ps[:, 0:SUP // 2])
                nc.vector.tensor_copy(
                    xt_sb[:, SUP // 2:SUP], xt_ps[:, SUP // 2:SUP]
                )
                xt_done[c] = xt_sb[:]
            if p >= 1:
                emit_mm(2 * p - 2)
                emit_mm(2 * p - 1)
            if p == 1 and pending_topk is not None:
                topk_state = emit_topk_early(*pending_topk)

        emit_mm(KC - 2)
        emit_mm(KC - 1)
        if pending_topk is not None:
            emit_topk_late(pending_topk[0], topk_state)
            pending_topk = None

        # zz = [L0 + b ; L1], both halves on partitions  [128, 512]
        z_sb = z_pool.tile([P, SUP], FP32, tag="z")
        nc.scalar.activation(
            z_sb[:], logits_ps[:],
            mybir.ActivationFunctionType.Identity, bias=b_sb[:],
        )
        pending_topk = (st, z_sb)

    if pending_topk is not None:
        topk_state = emit_topk_early(*pending_topk)
        emit_topk_late(pending_topk[0], topk_state)


_NC_CACHE = None


def _get_nc():
    global _NC_CACHE
    if _NC_CACHE is None:
        _NC_CACHE = build_nc()
    return _NC_CACHE


def run_sharded(hidden_states, W, b, trace=False):
    nc = _get_nc()
    hs = np.ascontiguousarray(np.asarray(hidden_states, dtype=np.float32))
    W = np.asarray(W, dtype=np.float32)
    b = np.ascontiguousarray(np.asarray(b, dtype=np.float32))
    # device wants W as [128, KC*64]: W[h=c*128+p, e] -> w_perm[p, c*64+e]
    w_perm = np.ascontiguousarray(
        W.reshape(KC, P, NUM_EXPERTS).transpose(1, 0, 2).reshape(P, KC * NUM_EXPERTS)
    )
    flat = hs.reshape(TOKENS, HIDDEN)
    in_maps = [
        {
            "x": flat[c * TOK_PER_CORE:(c + 1) * TOK_PER_CORE],
            "w": w_perm,
            "b": b,
        }
        for c in range(N_CORES)
    ]
    out = bass_utils.run_bass_kernel_spmd(
        nc, in_maps, core_ids=list(range(N_CORES)), trace=trace
    )
    return out


def _unperm(a, width):
    """[128, NG*width] partition-major -> [TOK_PER_CORE, width] token-major."""
    ng = a.shape[1] // width
    return a.reshape(P, ng, width).transpose(1, 0, 2).reshape(P * ng, width)


def kernel(hidden_states, W, b):
    out = run_sharded(hidden_states, W, b)
    res = out.results
    probs = np.concatenate([_unperm(r["probs"], TOPK) for r in res], axis=0)
    idx = np.concatenate([_unperm(r["idx"], TOPK) for r in res], axis=0)
    routing = np.concatenate(
        [_unperm(r["routing"], NUM_EXPERTS) for r in res], axis=0
    )
    probs_topk = probs.reshape(B, S, TOPK).astype(np.float32)
    indices_topk = idx.reshape(B, S, TOPK).astype(np.int32)
    routing_map = routing.reshape(B, S, NUM_EXPERTS).astype(np.float32)
    return probs_topk, indices_topk, routing_map
